# revision 11
# baseline (speedup 1.0000x reference)
"""ChannelGroupAttention kernel for Trainium2 (8 NeuronCores, SPMD).

Math: out[b, co, h, w] = sum_ci x[b, ci, h, w] * C[ci, co] with
C = repeat_interleave(G, 32, both axes).  Because C is block-constant,
this collapses to:
  T[b, go, hw]  = sum_ci Chat[ci, go] * x[b, ci, hw]   (Chat = repeat(G, 32, axis=0), [256, 8])
  out[b, co, :] = T[b, co // 32, :]                    (broadcast 8 rows -> 256 rows)

Per-core plan (data-parallel over batch, 4 batches/core):
  stage 1: PE matmul, K=256 via 2 accumulating 128-tiles.  The weight matrix
    replicates Chat's 8 columns at M-offsets {0,32,64,96} (zeros elsewhere), so
    T lands in PSUM at partitions {32r..32r+8} for all 4 row groups at no extra
    PE cost (matmul cost scales with streamed columns, not M).
  stage 2: 4-way row-group-packed matmuls (tile_position=(32r,0), K=8 0/1
    selector weights) broadcast T -> [128, n] chunks concurrently.
  DVE copies PSUM -> SBUF; 1.6MB DMAs both directions.
"""

import numpy as np

from concourse import bacc, mybir, tile
from concourse.bass_utils import run_bass_kernel_spmd

B, C_IN, H, W = 32, 256, 56, 56
HW = H * W  # 3136
NG = 8          # groups
SCALE = C_IN // NG  # 32
N_CORES = 8
B_PER = B // N_CORES  # 4 batches per core
NT = 448        # moving-operand tile (<=512 fp32 PSUM bank limit); 7*448 = 3136
N_TILES = HW // NT

FP32 = mybir.dt.float32

TRACE = False
LAST_RESULT = [None]

_compiled = [None]


def _build(repeats: int = 1):
    nc = bacc.Bacc("TRN2", target_bir_lowering=False, debug=False)

    x_d = nc.dram_tensor("x", [B_PER, 2, 128, HW], FP32, kind="ExternalInput")
    cw_d = nc.dram_tensor("cw", [2, 128, 128], FP32, kind="ExternalInput")
    bw_d = nc.dram_tensor("bw", [128, 2, 128], FP32, kind="ExternalInput")
    y_d = nc.dram_tensor("y", [B_PER, 2, 128, HW], FP32, kind="ExternalOutput")

    with tile.TileContext(nc) as tc:
        with (
            tc.tile_pool(name="wpool", bufs=1) as wpool,
            tc.tile_pool(name="xpool", bufs=4) as xpool,
            tc.tile_pool(name="tpool", bufs=3) as tpool,
            tc.tile_pool(name="opool", bufs=3) as opool,
            tc.tile_pool(name="ps1", bufs=2, space="PSUM") as ps1,
            tc.tile_pool(name="ps2", bufs=2, space="PSUM") as ps2,
        ):
            cw = wpool.tile([128, 2, 128], FP32, name="cw")
            nc.sync.dma_start(out=cw[:, 0, :], in_=cw_d[0])
            nc.sync.dma_start(out=cw[:, 1, :], in_=cw_d[1])
            bw = wpool.tile([128, 2, 128], FP32, name="bw")
            nc.sync.dma_start(out=bw[:], in_=bw_d[:])

            # Pipeline in slices of n (finer than a batch) to cut ramp-in.
            # in-DMAs ride the SP HWDGE ring, out-DMAs the ACT ring, so a
            # slot-blocked in-DMA can't head-of-line-block the outs.
            # PSUM tiles hold 2 banks ([128, 2, 512], 448 used per bank) so one
            # DVE copy drains two matmul n-tiles.
            SLICES = [(0, 2 * NT), (2 * NT, 4 * NT), (4 * NT, HW)]  # 896+896+1344
            ridx = 0
            for it, b in enumerate(
                bb for _ in range(repeats) for bb in range(B_PER)
            ):
                for s0, s1 in SLICES:
                    w = s1 - s0
                    n_sub = w // NT
                    groups = [(0, min(2, n_sub))]
                    while groups[-1][1] < n_sub:
                        g0 = groups[-1][1]
                        groups.append((g0, min(g0 + 2, n_sub)))

                    xt0 = xpool.tile([128, w], FP32, tag="xt0")
                    xt1 = xpool.tile([128, w], FP32, tag="xt1")
                    nc.sync.dma_start(out=xt0[:], in_=x_d[b, 0, :, s0:s1])
                    nc.sync.dma_start(out=xt1[:], in_=x_d[b, 1, :, s0:s1])

                    # stage 1: T replicated at psum partitions {32r..32r+8}
                    tsb = tpool.tile([128, w], FP32, tag="tsb")
                    for g0, g1 in groups:
                        gn = g1 - g0
                        pt = ps1.tile([128, gn, 512], FP32, tag="pt")
                        for n in range(g0, g1):
                            sl = slice(n * NT, (n + 1) * NT)
                            nc.tensor.matmul(
                                pt[:, n - g0, :NT], cw[:, 0, :], xt0[:, sl],
                                start=True, stop=False,
                            )
                            nc.tensor.matmul(
                                pt[:, n - g0, :NT], cw[:, 1, :], xt1[:, sl],
                                start=False, stop=True,
                            )
                        nc.vector.tensor_copy(
                            tsb[:, g0 * NT : g1 * NT], pt[:, :, :NT]
                        )

                    # stage 2: 4-way row-group packed broadcast matmuls
                    osb = [
                        opool.tile([128, w], FP32, tag=f"osb{m}",
                                   name=f"osb{m}_{it}_{s0}")
                        for m in range(2)
                    ]
                    for m in range(2):
                        for g0, g1 in groups:
                            gn = g1 - g0
                            po = ps2.tile([128, gn, 512], FP32, tag="po")
                            for n in range(g0, g1):
                                r = ridx % 4
                                ridx += 1
                                sl = slice(n * NT, (n + 1) * NT)
                                nc.tensor.matmul(
                                    po[:, n - g0, :NT],
                                    bw[32 * r : 32 * r + NG, m, :],
                                    tsb[32 * r : 32 * r + NG, sl],
                                    start=True,
                                    stop=True,
                                    tile_position=(32 * r, 0),
                                )
                            nc.vector.tensor_copy(
                                osb[m][:, g0 * NT : g1 * NT], po[:, :, :NT]
                            )
                        nc.scalar.dma_start(out=y_d[b, m, :, s0:s1], in_=osb[m][:])

    nc.compile()
    return nc


def build_in_maps(x: np.ndarray, G: np.ndarray) -> list:
    x = np.ascontiguousarray(x, dtype=np.float32)
    G = np.ascontiguousarray(G, dtype=np.float32)
    assert x.shape == (B, C_IN, H, W) and G.shape == (NG, NG)

    # stage-1 weights: W[h][k, m] = G[(h*128+k)//32, m%32] if m%32 < 8 else 0
    cw = np.zeros((2, 128, 128), dtype=np.float32)
    for h in range(2):
        for mo in range(4):
            cw[h, :, 32 * mo : 32 * mo + NG] = np.repeat(
                G[4 * h : 4 * h + 4, :], SCALE, axis=0
            )
    # stage-2 selector: bw[32r+g, m, p] = 1 iff g == 4m + p//32
    bw = np.zeros((128, 2, 128), dtype=np.float32)
    for r in range(4):
        for m in range(2):
            for p in range(128):
                g = 4 * m + p // SCALE
                if g < NG:
                    bw[32 * r + g, m, p] = 1.0

    xs = x.reshape(N_CORES, B_PER, 2, 128, HW)
    return [
        {"x": np.ascontiguousarray(xs[i]), "cw": cw, "bw": bw}
        for i in range(N_CORES)
    ]


def kernel(x: np.ndarray, G: np.ndarray) -> np.ndarray:
    if _compiled[0] is None:
        _compiled[0] = _build()
    nc = _compiled[0]

    in_maps = build_in_maps(x, G)
    res = run_bass_kernel_spmd(nc, in_maps, core_ids=list(range(N_CORES)), trace=TRACE)
    LAST_RESULT[0] = res

    out = np.concatenate([res.results[i]["y"].reshape(B_PER, C_IN, H, W)
                          for i in range(N_CORES)], axis=0)
    return out


# revision 17
# speedup vs baseline: 1.6806x; 1.6806x over previous
"""ChannelGroupAttention kernel for Trainium2 (8 NeuronCores, SPMD).

Math: out[b, co, h, w] = sum_ci x[b, ci, h, w] * C[ci, co] with
C = repeat_interleave(G, 32, both axes).  Because C is block-constant,
this collapses to:
  T[b, go, hw]  = sum_ci Chat[ci, go] * x[b, ci, hw]   (Chat = repeat(G, 32, axis=0), [256, 8])
  out[b, co, :] = T[b, co // 32, :]                    (broadcast 8 rows -> 256 rows)

Per-core plan (data-parallel over batch, 4 batches/core):
  stage 1: PE matmul, K=256 via 2 accumulating 128-row tiles (fp32, exact).
    The weight matrix places T[k] at psum partition 32k and T[4+k] at 32k+1
    (other M columns zero) at no extra PE cost (matmul cost scales with
    streamed columns, not M).
  stage 2: DVE stream_shuffle with a uniform mask broadcasts partition 32k
    (32k+1) across each 32-partition block, materializing output chunk 0 (1)
    straight from PSUM into SBUF - no second matmul, no extra copies.
  ~0.5-0.9MB DMAs both directions; in-DMAs on the SP HWDGE ring, out-DMAs on
  the ACT ring.  Measured at the pure-DMA roofline (~78us/core vs ~80us floor
  for the 25.7MB/core round trip).
"""

import numpy as np

from concourse import bacc, mybir, tile
from concourse.bass_utils import run_bass_kernel_spmd

B, C_IN, H, W = 32, 256, 56, 56
HW = H * W  # 3136
NG = 8          # groups
SCALE = C_IN // NG  # 32
N_CORES = 8
B_PER = B // N_CORES  # 4 batches per core
NT = 448        # moving-operand tile (<=512 fp32 PSUM bank limit); 7*448 = 3136
N_TILES = HW // NT

FP32 = mybir.dt.float32

TRACE = False
LAST_RESULT = [None]

_compiled = [None]


def _build(repeats: int = 1):
    """v5: stage 1 matmul + DVE stream_shuffle broadcast (no stage-2 matmul).

    Stage-1 weights place T[k] at psum partition 32k and T[4+k] at partition
    32k+1.  stream_shuffle with an all-0 (all-1) mask then broadcasts within
    each 32-partition block, materializing output chunk 0 (1) directly.
    """
    nc = bacc.Bacc("TRN2", target_bir_lowering=False, debug=False)

    x_d = nc.dram_tensor("x", [B_PER, 2, 128, HW], FP32, kind="ExternalInput")
    cw_d = nc.dram_tensor("cw", [2, 128, 128], FP32, kind="ExternalInput")
    y_d = nc.dram_tensor("y", [B_PER, 2, 128, HW], FP32, kind="ExternalOutput")

    with tile.TileContext(nc) as tc:
        with (
            tc.tile_pool(name="wpool", bufs=1) as wpool,
            tc.tile_pool(name="xpool", bufs=4) as xpool,
            tc.tile_pool(name="opool", bufs=3) as opool,
            tc.tile_pool(name="ps1", bufs=4, space="PSUM") as ps1,
        ):
            cw = wpool.tile([128, 2, 128], FP32, name="cw")
            nc.sync.dma_start(out=cw[:, 0, :], in_=cw_d[0])
            nc.sync.dma_start(out=cw[:, 1, :], in_=cw_d[1])

            SLICES = [(0, 2 * NT), (2 * NT, 4 * NT), (4 * NT, HW)]  # 896+896+1344
            for it, b in enumerate(
                bb for _ in range(repeats) for bb in range(B_PER)
            ):
                for s0, s1 in SLICES:
                    w = s1 - s0
                    n_sub = w // NT
                    groups = [(0, min(2, n_sub))]
                    while groups[-1][1] < n_sub:
                        g0 = groups[-1][1]
                        groups.append((g0, min(g0 + 2, n_sub)))

                    xt0 = xpool.tile([128, w], FP32, tag="xt0")
                    xt1 = xpool.tile([128, w], FP32, tag="xt1")
                    nc.sync.dma_start(out=xt0[:], in_=x_d[b, 0, :, s0:s1])
                    nc.sync.dma_start(out=xt1[:], in_=x_d[b, 1, :, s0:s1])

                    osb = [
                        opool.tile([128, w], FP32, tag=f"osb{m}",
                                   name=f"osb{m}_{it}_{s0}")
                        for m in range(2)
                    ]
                    for g0, g1 in groups:
                        gn = g1 - g0
                        pt = ps1.tile([128, gn, 512], FP32, tag="pt")
                        for n in range(g0, g1):
                            sl = slice(n * NT, (n + 1) * NT)
                            nc.tensor.matmul(
                                pt[:, n - g0, :NT], cw[:, 0, :], xt0[:, sl],
                                start=True, stop=False,
                            )
                            nc.tensor.matmul(
                                pt[:, n - g0, :NT], cw[:, 1, :], xt1[:, sl],
                                start=False, stop=True,
                            )
                        for m in range(2):
                            for i in range(gn):
                                nc.vector.stream_shuffle(
                                    osb[m][:, (g0 + i) * NT : (g0 + i + 1) * NT],
                                    pt[:, i, :NT],
                                    mask=[m] * 32,
                                )
                    for m in range(2):
                        nc.scalar.dma_start(out=y_d[b, m, :, s0:s1], in_=osb[m][:])

    nc.compile()
    return nc


def build_in_maps(x: np.ndarray, G: np.ndarray) -> list:
    x = np.ascontiguousarray(x, dtype=np.float32)
    G = np.ascontiguousarray(G, dtype=np.float32)
    assert x.shape == (B, C_IN, H, W) and G.shape == (NG, NG)

    # stage-1 weights: psum partition 32k gets T[k] (output chunk 0, block k),
    # psum partition 32k+1 gets T[4+k] (chunk 1, block k).
    # cw[h][ci, 32k + m] = Chat[128h + ci, 4m + k] = G[(128h+ci)//32, 4m + k]
    chat_h = np.repeat(G, SCALE, axis=0).reshape(2, 128, NG)  # [h, ci, go]
    cw = np.zeros((2, 128, 128), dtype=np.float32)
    for k in range(4):
        for m in range(2):
            cw[:, :, 32 * k + m] = chat_h[:, :, 4 * m + k]

    xs = x.reshape(N_CORES, B_PER, 2, 128, HW)
    return [
        {"x": np.ascontiguousarray(xs[i]), "cw": cw}
        for i in range(N_CORES)
    ]


def kernel(x: np.ndarray, G: np.ndarray) -> np.ndarray:
    if _compiled[0] is None:
        _compiled[0] = _build()
    nc = _compiled[0]

    in_maps = build_in_maps(x, G)
    res = run_bass_kernel_spmd(nc, in_maps, core_ids=list(range(N_CORES)), trace=TRACE)
    LAST_RESULT[0] = res

    out = np.concatenate([res.results[i]["y"].reshape(B_PER, C_IN, H, W)
                          for i in range(N_CORES)], axis=0)
    return out
